# revision 59
# baseline (speedup 1.0000x reference)
"""Trainium2 Bass kernel for the DCE 2CXM signal model — log-folded 4-channel
exp basis, reciprocal epilogue.

Math per pixel: theta_m/theta_p from the 2CXM params; conc[k] =
a1*Gk(thm) + a2*Gk(thp) + u*(Gk(thm) - Gk(thp)) with a1, a2, u all > 0
(a1 = vp*alpha/(alpha*Sm+beta*Sp), a2 = vp*beta/(...), u = ve/(Sm-Sp)).
Gk(th) = sum_t A[k,t] exp(-0.1 t th) is fitted per channel with 16 (or 15)
exponential nodes on the channel's empirical theta range.  The positive
coefficients are folded into the exponentials via logs:
c*exp(-0.1 s th) = exp(-0.1 s th + ln c), so one broadcast matmul (mm1,
K=10) builds all 128 exp arguments per column (2 pixels/column, 64
partitions each: 16 a1|m + 16 a2|p + 16 u|m + 15 u|p + 1 zero -> exp=1),
one ACT Exp evaluates the basis, and one block-diagonal matmul (mm2,
K=128 -> M=100) contracts straight to P' = (VH0 + s*conc)/K2p for both
pixel halves at once (the exp(0)=1 row carries the VH0 bias).  The SPGR
epilogue uses coth(v) ~= 1/v (abs err < 0.04 on a ~300 signal):
sig = K1p + K2p/v, i.e. one DVE reciprocal per output element.  The
device emits R = K2p/v in fp32; the host adds K1p during output
assembly.
"""

import os
from contextlib import ExitStack

import numpy as np

H = W = 320
NPIX = H * W
NCORES = 8
SHARD = NPIX // NCORES      # 12800 pixels per core
HALF = SHARD // 2           # 6400   (2 pixels per basis column)
PC = 100                    # prep layout [128, 100]
TS = 50
STEP = 0.1
DELAY = 30
LF = 589
SC = 1024                   # superchunk columns

SIG_BASELINE = 100.0
R1 = 1.0
R1CA = 4.3
FA = 10.0
TR = 0.00487

_CACHE: dict = {}


def _spgr_consts():
    f32 = np.float32
    fa = FA * np.pi / 180.0
    cosf = float(np.cos(f32(fa)))
    sinf = float(np.sin(f32(fa)))
    E1 = float(np.exp(f32(-TR * R1)))
    M0 = SIG_BASELINE * (1.0 - cosf * E1) / (sinf * (1.0 - E1))
    M0t = M0 * sinf
    M_st = M0t * (1.0 - E1) / (1.0 - E1 * cosf)
    C0 = SIG_BASELINE - M_st
    K1 = C0 + M0t / cosf
    K2 = M0t * (cosf - 1.0) / cosf
    K1p = K1 + K2 / 2.0
    K2p = -K2 / 2.0
    VH0 = 0.5 * (-TR * R1 + np.log(cosf))
    SS = -TR * R1CA / 2.0
    return K1p, K2p, VH0, SS


def _patch_act_tables():
    """Make Exp/Ln/Copy resolve only to natural_log_exp_and_others so the
    table-load pass emits a single load instead of ping-ponging between
    exp_and_others and natural_log_exp_and_others (1.3us per switch)."""
    import concourse.bacc as bacc_mod
    from concourse import mybir
    from concourse.hw_specs import get_activation_tables as _orig

    AF = mybir.ActivationFunctionType
    mine = {AF.Exp, AF.Ln, AF.Copy, AF.Identity}

    def patched(arch):
        tabs = _orig(arch)
        out = {}
        for name, fns in tabs.items():
            if name == "natural_log_exp_and_others":
                out[name] = set(fns) | {AF.Copy, AF.Identity}
            else:
                out[name] = set(fns) - mine
        return out

    bacc_mod.get_activation_tables = patched


def _build_bass():
    import concourse.bass as bass
    import concourse.tile as tile
    from concourse import bacc, mybir

    _patch_act_tables()

    f32 = mybir.dt.float32
    f32r = mybir.dt.float32r
    f16 = mybir.dt.float16
    AF = mybir.ActivationFunctionType
    ALU = mybir.AluOpType

    nc = bacc.Bacc()
    pmap = nc.dram_tensor("pmap", [4, SHARD], f32, kind="ExternalInput")
    wall = nc.dram_tensor("wall", [128, PC + 128], f32, kind="ExternalInput")
    sig = nc.dram_tensor("sig", [TS, SHARD], f32, kind="ExternalOutput")

    K1p, K2p, VH0, SS = _spgr_consts()
    NWARM = int(os.environ.get("DCE_WARM", "0"))

    with tile.TileContext(nc) as tc, ExitStack() as ctx:
        const = ctx.enter_context(tc.tile_pool(name="const", bufs=1))
        thps = ctx.enter_context(
            tc.tile_pool(name="thps", bufs=2, space=bass.MemorySpace.PSUM))
        pps = ctx.enter_context(
            tc.tile_pool(name="pps", bufs=2, space=bass.MemorySpace.PSUM))
        baspool = ctx.enter_context(tc.tile_pool(name="bas", bufs=4))
        rpool = ctx.enter_context(tc.tile_pool(name="rp", bufs=6))
        rows = ctx.enter_context(tc.tile_pool(name="rows", bufs=1))
        prep = ctx.enter_context(tc.tile_pool(name="prep", bufs=1))

        V = nc.vector
        G = nc.gpsimd
        SCL = nc.scalar

        # input DMAs first (they gate the prep chain); fp, then ps, then
        # ve+vp so the chain head (recip(fp), recip(ps)) starts earliest.
        pin = prep.tile([128, 4 * PC], f32, tag="pin", name="pin")
        ve = pin[:, 0 * PC:1 * PC]
        vp = pin[:, 1 * PC:2 * PC]
        fp_ = pin[:, 2 * PC:3 * PC]
        ps_ = pin[:, 3 * PC:4 * PC]
        nc.sync.dma_start(
            out=fp_, in_=pmap[2, :].rearrange("(p c) -> p c", p=128))
        nc.sync.dma_start(
            out=ps_, in_=pmap[3, :].rearrange("(p c) -> p c", p=128))
        nc.sync.dma_start(
            out=pin[:, 0:2 * PC].rearrange("p (i c) -> p i c", i=2),
            in_=pmap[0:2, :].rearrange("i (p c) -> p i c", p=128))

        # Tiny memset+exp: starts the (single) ACT table load for
        # natural_log_exp at t~0, overlapped with the input DMAs.
        tl = const.tile([1, 1], f32, tag="tl", name="tl")
        V.memset(tl, 0.0)
        SCL.activation(tl, tl, AF.Exp, bias=0.0, scale=1.0)

        wtile_f = const.tile([128, PC + 128], f32, tag="wallf", name="wallf")
        nc.sync.dma_start(out=wtile_f, in_=wall[:])
        wtile = const.tile([128, PC + 128], f32r, tag="wallr", name="wallr")
        V.tensor_copy(wtile, wtile_f)
        lhsT2 = wtile[:, 0:PC]
        lhsT1 = wtile[0:10, PC:PC + 128]

        # PE warm-up: garbage matmuls during prep so the HAM ramp (~3us)
        # completes before the real main-loop matmuls.
        for wi in range(NWARM):
            wt = thps.tile([128, SC], f32, tag="th_ps", name=f"warm{wi}")
            nc.tensor.matmul(wt[0:PC, 0:PC], lhsT2, lhsT2[:, 0:PC],
                             start=True, stop=True)

        # ---------------- prep: pixel-major [128, 100] ----------------
        def pt(tag, dt=f32):
            return prep.tile([128, PC], dt, tag=tag, name=tag)

        out5 = prep.tile([128, 5 * PC], f32r, tag="out5", name="out5")
        thm_h = out5[:, 0 * PC:1 * PC]
        thp_h = out5[:, 1 * PC:2 * PC]
        lna1 = out5[:, 2 * PC:3 * PC]
        lna2 = out5[:, 3 * PC:4 * PC]
        lnu = out5[:, 4 * PC:5 * PC]

        # Critical chain lives on DVE; Pool handles off-chain branches; ACT
        # only for ln/exp. Cross-engine hops cost ~0.5us, same-engine ~0.25.
        rfp = pt("rfp"); V.reciprocal_approx_fast(rfp, fp_)
        rps = pt("rps"); V.reciprocal_approx_fast(rps, ps_)
        Te = pt("Te"); V.tensor_mul(Te, ve, rps)
        Tc = pt("Tc"); V.tensor_mul(Tc, vp, rfp)
        svp = pt("svp"); G.tensor_add(svp, vp, ve)       # off-chain (Pool)
        T_ = pt("T_"); V.tensor_mul(T_, svp, rfp)
        S_ = pt("S_"); V.tensor_add(S_, T_, Te)
        TcTe = pt("TcTe"); V.tensor_mul(TcTe, Tc, Te)
        S2 = pt("S2")
        V.scalar_tensor_tensor(S2, S_, 1.0, S_, op0=ALU.mult, op1=ALU.mult)
        d2 = pt("d2")
        V.scalar_tensor_tensor(d2, TcTe, -4.0, S2, op0=ALU.mult, op1=ALU.add)
        lnd = pt("lnd"); SCL.activation(lnd, d2, AF.Ln, bias=0.0, scale=1.0)
        disc = pt("disc")
        SCL.activation(disc, lnd, AF.Exp, bias=0.0, scale=0.5)
        # during the ACT detour, DVE computes rTT (needed right after)
        rTT = pt("rTT"); V.reciprocal_approx_fast(rTT, TcTe)
        den_ = pt("den"); V.tensor_add(den_, S_, disc)
        thmt = pt("thmt"); V.reciprocal_approx_fast(thmt, den_)
        thpt = pt("thpt"); V.tensor_mul(thpt, den_, rTT)

        r1m = pt("r1m")
        SCL.activation(r1m, thmt, AF.Exp, bias=0.0, scale=-0.2)
        r1p = pt("r1p")
        SCL.activation(r1p, thpt, AF.Exp, bias=0.0, scale=-0.05)
        rlm = pt("rlm")
        SCL.activation(rlm, thmt, AF.Exp, bias=0.0, scale=-0.2 * LF)

        # off-chain (Pool) while ACT runs: alpha/beta pieces and f32r copies
        alt = pt("alt")
        G.tensor_scalar_mul(alt, Te, -2.0)
        al0 = pt("al0"); G.tensor_mul(al0, alt, thmt)
        al = pt("al"); G.tensor_scalar_add(al, al0, 1.0)
        btt = pt("btt"); G.tensor_scalar_mul(btt, Te, 0.5)
        bt0 = pt("bt0"); G.tensor_mul(bt0, btt, thpt)
        bt = pt("bt"); G.tensor_scalar_sub(bt, bt0, 1.0)
        G.tensor_copy(thm_h, thmt)
        G.tensor_copy(thp_h, thpt)

        # DVE run 2: finish u first (so its rows DMAs launch earliest),
        # then a2, then c1-side.
        Dm = pt("Dm"); V.tensor_scalar(Dm, r1m, -1.0, 1.0, op0=ALU.mult, op1=ALU.add)
        Dp = pt("Dp"); G.tensor_scalar(Dp, r1p, -1.0, 1.0, op0=ALU.mult, op1=ALU.add)
        Nm = pt("Nm"); V.tensor_scalar(Nm, rlm, -1.0, 1.0, op0=ALU.mult, op1=ALU.add)
        P1 = pt("P1"); V.tensor_mul(P1, Nm, Dp)
        dd = pt("dd"); V.tensor_sub(dd, P1, Dm)
        rdd = pt("rdd"); V.reciprocal_approx_fast(rdd, dd)
        W_ = pt("W_"); G.tensor_mul(W_, Dm, Dp)           # Pool, off-chain
        veW = pt("veW"); G.tensor_mul(veW, ve, W_)        # Pool
        vpW = pt("vpW"); G.tensor_mul(vpW, vp, W_)        # Pool
        ut = pt("ut"); V.tensor_mul(ut, veW, rdd)
        SCL.activation(lnu, ut, AF.Ln, bias=0.0, scale=1.0)
        aP1 = pt("aP1"); V.tensor_mul(aP1, al, P1)
        bP2 = pt("bP2"); V.tensor_mul(bP2, bt, Dm)
        den1 = pt("den1"); V.tensor_add(den1, aP1, bP2)
        r1_ = pt("r1_"); V.reciprocal_approx_fast(r1_, den1)
        t5 = pt("t5"); G.tensor_mul(t5, vpW, bt)          # Pool, ready early
        a2t = pt("a2t"); V.tensor_mul(a2t, t5, r1_)
        SCL.activation(lna2, a2t, AF.Ln, bias=0.0, scale=1.0)
        t4 = pt("t4"); G.tensor_mul(t4, vpW, al)          # Pool, ready early
        a1t = pt("a1t"); V.tensor_mul(a1t, t4, r1_)
        SCL.activation(lna1, a1t, AF.Ln, bias=0.0, scale=1.0)

        # rows [10, HALF]: partitions 0:4 = theta block (thmA, thpA, thmB,
        # thpB), 4:10 = log block (lnc1A, lna2A, lnuA, lnc1B, lna2B, lnuB).
        # Issue order: theta pairs first (ready early), then lnu, lna2, lna1.
        rows_t = rows.tile([10, HALF], f32r, tag="rows", name="rows")
        ROWMAP = {(0, 0): 0, (1, 0): 1, (0, 1): 2, (1, 1): 3,
                  (2, 0): 4, (3, 0): 5, (4, 0): 6,
                  (2, 1): 7, (3, 1): 8, (4, 1): 9}
        for v in (0, 1, 4, 3, 2):
            # late-completing values (lna2 v=3, lna1 v=2) go through Pool
            # SWDGE so they run parallel to the HWDGE queue draining lnu.
            eng = G if v == 2 else nc.sync
            for h in range(2):
                r = ROWMAP[(v, h)]
                eng.dma_start(
                    out=rows_t[r: r + 1, :],
                    in_=out5[h * 64:(h + 1) * 64, v * PC:(v + 1) * PC])

        # PE warm-up bridge: after the theta rows land, run junk matmuls on
        # them so the PE HAM ramp is complete when the real loop starts.
        NWARM2 = int(os.environ.get("DCE_WARM2", "18"))
        lhsT_w = wtile[0:4, PC:PC + 128]
        for wi in range(NWARM2):
            wt = thps.tile([128, SC], f32, tag="th_ps", name=f"wb{wi}")
            nc.tensor.matmul(wt[:, 0:512], lhsT_w, rows_t[0:4, 0:512],
                             start=True, stop=True)

        # ---------------- main loop ----------------
        # Superchunks of SC cols; output DMAs batched per PAIR of
        # superchunks ([100, 2*SC] wide) to halve DMA count.
        n_sc = (HALF + SC - 1) // SC     # 7 (6x1024 + 1x256)
        rt2 = None
        for isc in [n_sc - 1] + list(range(n_sc - 1)):
            base = isc * SC
            cols = min(SC, HALF - base)
            th_ps = thps.tile([128, SC], f32, tag="th_ps", name=f"th_ps{isc}")
            for o in range(0, cols, 512):
                w = min(512, cols - o)
                nc.tensor.matmul(
                    th_ps[:, o:o + w], lhsT1,
                    rows_t[0:10, base + o:base + o + w],
                    start=True, stop=True)
            bas = baspool.tile([128, SC], f32r, tag="bas", name=f"bas{isc}")
            SCL.activation(bas[:, :cols], th_ps[:, :cols], AF.Exp,
                           bias=0.0, scale=1.0)
            p_ps = pps.tile([100, SC], f32, tag="p_ps", name=f"p_ps{isc}")
            for o in range(0, cols, 512):
                w = min(512, cols - o)
                nc.tensor.matmul(
                    p_ps[:, o:o + w], lhsT2,
                    bas[:, o:o + w],
                    start=True, stop=True)
            # First 4 superchunks: paired [100, 2*SC] outputs (fewer DMAs
            # while HWDGE is busy). Last 3: individual DMAs so the tail
            # transfers start as soon as each reciprocal lands.
            if isc < 4:
                half = isc % 2
                pair = isc // 2
                if half == 0:
                    rt2 = rpool.tile([100, 2 * SC], f32, tag="rt2",
                                     name=f"rt2_{pair}")
                V.reciprocal_approx_fast(
                    rt2[:, half * SC:half * SC + cols], p_ps[:, :cols])
                if half == 1:
                    pbase = pair * 2 * SC
                    for h in range(2):
                        nc.sync.dma_start(
                            out=sig[:, h * HALF + pbase:
                                    h * HALF + pbase + 2 * SC],
                            in_=rt2[h * TS:(h + 1) * TS, :])
            else:
                rts = rpool.tile([100, SC], f32, tag="rts", name=f"rts{isc}")
                V.reciprocal_approx_fast(rts[:, :cols], p_ps[:, :cols])
                for h in range(2):
                    nc.sync.dma_start(
                        out=sig[:, h * HALF + base: h * HALF + base + cols],
                        in_=rts[h * TS:(h + 1) * TS, :cols])

    nc.compile()
    return nc


def _host_prep(param: np.ndarray, sample_time: np.ndarray, Cp: np.ndarray):
    """AIF conv matrix + per-channel exponential-sum fits -> lhsT1/lhsT2."""
    f32 = np.float32
    t32 = np.arange(LF, dtype=f32) * f32(STEP)
    aifci = np.interp(
        t32.astype(np.float64),
        np.asarray(sample_time, np.float64),
        np.asarray(Cp, np.float64))
    aif = np.concatenate([np.zeros(DELAY), aifci[:-DELAY]])
    idx = np.minimum(
        np.searchsorted(t32, np.asarray(sample_time, f32), side="left"),
        LF - 1)
    A = np.zeros((TS, LF))
    for k in range(TS):
        i = int(idx[k])
        A[k, : i + 1] = aif[i::-1]

    # empirical theta ranges (cheap fp64 host pass over the param maps)
    ve, vp, fp_, ps_ = [np.asarray(param[i], np.float64).ravel()
                        for i in range(4)]
    Te = ve / ps_
    S = (vp + ve) / fp_ + Te
    TcTe = (vp / fp_) * Te
    disc = np.sqrt(S * S - 4.0 * TcTe)
    thm = 2.0 / (S + disc)
    thp = (S + disc) / (2.0 * TcTe)
    tm = (float(thm.min()) * 0.98, float(thm.max()) * 1.02)
    tp = (float(thp.min()) * 0.98, float(thp.max()) * 1.02)

    def fit(nodes, tlo, thi, ngrid=3000, lam_rel=1e-10):
        th = np.geomspace(tlo, thi, ngrid)
        Eg = np.exp(-STEP * np.outer(th, np.arange(LF)))
        F = Eg @ A.T
        B = np.exp(-STEP * np.outer(th, nodes))
        lam = lam_rel * np.linalg.norm(B, 2) ** 2
        return np.linalg.solve(B.T @ B + lam * np.eye(len(nodes)), B.T @ F)

    nm16 = np.concatenate([[0.0], np.geomspace(0.5, 588.0, 15)])
    np16 = np.concatenate([[0.0], np.geomspace(0.1, 588.0, 15)])
    np15 = np.concatenate([[0.0], np.geomspace(0.1, 588.0, 14)])
    Cm = fit(nm16, *tm)      # [16, TS]
    Cq = fit(np16, *tp)      # [16, TS]
    Cq15 = fit(np15, *tp)    # [15, TS]

    K1p, K2p, VH0, SS = _spgr_consts()
    q = SS / K2p

    wall = np.zeros((128, PC + 128), f32)
    w2 = wall[:, 0:PC]
    w1 = wall[0:10, PC:PC + 128]
    for h in range(2):
        b = h * 64
        k0 = h * TS
        w2[b + 0:b + 16, k0:k0 + TS] = (q * Cm).astype(f32)
        w2[b + 16:b + 32, k0:k0 + TS] = (q * Cq).astype(f32)
        w2[b + 32:b + 48, k0:k0 + TS] = (q * Cm).astype(f32)
        w2[b + 48:b + 63, k0:k0 + TS] = (-q * Cq15).astype(f32)
        w2[b + 63, k0:k0 + TS] = f32(VH0 / K2p)
        r0 = h * 5
        # rows layout: 0-3 = theta block (thmA, thpA, thmB, thpB),
        # 4-9 = log block (lna1A, lna2A, lnuA, lna1B, lna2B, lnuB)
        r_thm = 0 + 2 * h
        r_thp = 1 + 2 * h
        r_ln1 = 4 + 3 * h
        r_ln2 = 5 + 3 * h
        r_lnu = 6 + 3 * h
        w1[r_thm, b + 0:b + 16] = (-0.2 * nm16).astype(f32)
        w1[r_thm, b + 32:b + 48] = (-0.2 * nm16).astype(f32)
        w1[r_thp, b + 16:b + 32] = (-0.05 * np16).astype(f32)
        w1[r_thp, b + 48:b + 63] = (-0.05 * np15).astype(f32)
        w1[r_ln1, b + 0:b + 16] = 1.0
        w1[r_ln2, b + 16:b + 32] = 1.0
        w1[r_lnu, b + 32:b + 63] = 1.0
    return wall


def kernel(param: np.ndarray, sample_time: np.ndarray, Cp: np.ndarray) -> np.ndarray:
    from concourse.bass_utils import run_bass_kernel_spmd

    if "nc" not in _CACHE:
        _CACHE["nc"] = _build_bass()
    nc = _CACHE["nc"]

    wall = _host_prep(param, sample_time, Cp)
    pflat = np.ascontiguousarray(np.asarray(param, np.float32).reshape(4, NPIX))
    in_maps = []
    for c in range(NCORES):
        in_maps.append({
            "pmap": np.ascontiguousarray(pflat[:, c * SHARD:(c + 1) * SHARD]),
            "wall": wall,
        })
    ncr = int(os.environ.get("DCE_CORES", str(NCORES)))
    res = run_bass_kernel_spmd(
        nc, in_maps[:ncr], core_ids=list(range(ncr)),
        trace=bool(int(os.environ.get("DCE_TRACE", "0"))),
    )
    if res.exec_time_ns is not None:
        _CACHE["exec_time_ns"] = res.exec_time_ns
    K1p = np.float32(_spgr_consts()[0])
    outs = [r["sig"] for r in res.results]
    while len(outs) < NCORES:
        outs.append(np.zeros((TS, SHARD), np.float32))
    out = np.concatenate(outs, axis=1) + K1p
    return out.reshape(TS, 1, H, W)
